# revision 1
# baseline (speedup 1.0000x reference)
"""CapsuleLayer (dynamic routing) Trainium2 Bass kernel.

Sharding: pure data-parallel over batch B=256 -> 8 cores x 32 batches.
Per core the 32 batches run as 4 sub-chunks of 8; the SBUF partition dim
packs p = b*16 + ig where capsule index i = 16*g + ig, g in [0,72).

Phase 1 (u_hat = einsum('nidk,bik->bnid')): the K=8 contraction is packed
to K=128 by block-diagonalizing 16 capsules' inputs into the PE stationary
operand (assembled on-chip from a compact masked load; far-off-diagonal
zeros are memset once):
    lhsT[(ig,k), (b',ig')] = x[b', 16g+ig', k] * (ig==ig')
    rhs  = W2[(ig,k), (g,d,n)] = W[n, 16g+ig, d, k]
    psum[(b,ig), (d,n)] = u_hat[b, n, 16g+ig, d]
u_hat stays on-chip in bf16 as U[128, G, D, N] (n innermost so broadcast
multiplies keep unit stride for the DVE 2x mode).

Routing (3 iters, fused on-chip):
  outputs-einsum: DVE multiply (c bcast over d) + PE partition-reduce with a
  fixed 0/1 bmask stationary, PSUM-accumulating 3 groups per matmul (F=480).
  agreement: DVE multiply (outputs bcast over g) + fold-tree d-reduction on
  GPSIMD. softmax over n: ACT exp + DVE reduce (logits bounded, no max-sub).

Emission is step-major (lockstep) across the 4 sub-chunks: engines execute
their streams in order, so per-sub-chunk emission would serialize the whole
routing chain; lockstep keeps each engine's queue dense.
"""

import numpy as np
import ml_dtypes

B, N, I, D, DK = 256, 10, 1152, 16, 8
NCORES = 8
BC = B // NCORES      # 32 batches per core
BS = 8                # batches per sub-chunk
NSUB = BC // BS       # 4
IG = 16               # capsules per PE group
G = I // IG           # 72
ND = D * N            # 160, (d-major, n-minor)
GBLK = 12             # g per routing block
NBLK = G // GBLK      # 6
MMB = 3               # groups PSUM-accumulated per routing matmul (F=480)
XSL = G * BS * 4 + 4  # per-sub-chunk xc slice (values + dmask tail)
BF16 = ml_dtypes.bfloat16

_cache = {}


def _bcast(ap, axis, count):
    """Insert a stride-0 dim of size `count` at `axis`."""
    ap = ap.unsqueeze(axis)
    shape = list(ap.shape)
    shape[axis] = count
    return ap.broadcast_to(shape)


def _legalize_waits(nc):
    """This walrus build takes at most 1 embedded sync wait per TPB
    instruction (2 on EventSemaphore, 0 on Drain). Tile emits multi-wait
    sync_info; hoist the extras onto preceding EventSemaphore instructions
    on the same engine queue."""
    from concourse import mybir

    n = 0
    for fn in nc.m.functions:
        for blk in fn.blocks:
            out = []
            for inst in blk.instructions:
                si = inst.sync_info
                if si is not None and si.on_wait:
                    keep = 1
                    if inst.opcode == "Drain":
                        keep = 0
                    elif inst.opcode == "EventSemaphore":
                        keep = 2
                    w = list(si.on_wait)
                    if len(w) > keep:
                        extra = w[:len(w) - keep] if keep else w
                        kept = w[len(w) - keep:] if keep else []
                        for i0 in range(0, len(extra), 2):
                            n += 1
                            out.append(mybir.InstEventSemaphore(
                                name=f"{inst.name}-hw{n}",
                                engine=inst.engine, ins=[], outs=[],
                                sync_info=mybir.SyncInfo(
                                    on_wait=extra[i0:i0 + 2],
                                    on_update=[]),
                            ))
                        si.on_wait = kept
                out.append(inst)
            blk.instructions = out
    return n


def _build_nc():
    import concourse.bass as bass
    import concourse.tile as tile
    from concourse import mybir
    from contextlib import ExitStack

    f32 = mybir.dt.float32
    bf16 = mybir.dt.bfloat16
    AX = mybir.AxisListType
    OP = mybir.AluOpType
    AF = mybir.ActivationFunctionType

    nc = bass.Bass()
    xc_d = nc.dram_tensor("xc", [128, NSUB, XSL], bf16, kind="ExternalInput")
    w2_d = nc.dram_tensor("w2", [128, G * ND], bf16, kind="ExternalInput")
    bmask_d = nc.dram_tensor("bmask", [128, BS], bf16, kind="ExternalInput")
    bcmask_d = nc.dram_tensor("bcmask", [BS, 128], bf16, kind="ExternalInput")
    y_d = nc.dram_tensor("y", [NSUB, BS, ND], f32, kind="ExternalOutput")

    with tile.TileContext(nc) as tc:
        with ExitStack() as ctx:
            singles = ctx.enter_context(tc.tile_pool(name="singles", bufs=1))
            upool = ctx.enter_context(tc.tile_pool(name="upool", bufs=4))

            GQ = G // 4
            w2q = []
            for q in range(4):
                w2t = singles.tile([128, GQ * ND], bf16, tag=f"w2_{q}")
                nc.sync.dma_start(
                    w2t, w2_d[:, q * GQ * ND:(q + 1) * GQ * ND])
                w2q.append(w2t)
            bmask = singles.tile([128, BS], bf16)
            nc.sync.dma_start(bmask, bmask_d[:])
            bcmask = singles.tile([BS, 128], bf16)
            nc.sync.dma_start(bcmask, bcmask_d[:])

            Us = []
            # ---------------- Phase A: u_hat build ----------------
            with ExitStack() as actx:
                xcpool = actx.enter_context(
                    tc.tile_pool(name="xcpool", bufs=1))
                ph1ps = actx.enter_context(
                    tc.tile_pool(name="ph1ps", bufs=6, space="PSUM"))

                xcs = []
                for s in range(NSUB):
                    xct = xcpool.tile([128, XSL], bf16, tag=f"xc{s}")
                    nc.sync.dma_start(xct, xc_d[:, s])
                    xcs.append(xct)
                xblk0 = xcpool.tile([128, G, BS, IG], bf16, tag="xblk0")
                nc.gpsimd.memset(xblk0, 0.0)
                xblk1 = xcpool.tile([128, G, BS, IG], bf16, tag="xblk1")
                nc.gpsimd.memset(xblk1, 0.0)
                xblks = [xblk0, xblk1]

                for s in range(NSUB):
                    xb = xblks[s % 2]
                    xv = xcs[s][:, :G * BS * 4]
                    dmk = xcs[s][:, G * BS * 4:]
                    # near-diagonal refresh: chunk [32j,32j+32) holds
                    # capsules ig in [4j,4j+4); dmask re-zeroes off-columns
                    for j in range(4):
                        mk = _bcast(_bcast(dmk[32 * j:32 * (j + 1)], 1, G),
                                    2, BS)
                        xvj = bass.AP(
                            tensor=xv.tensor, offset=xv.offset,
                            ap=list(xv.ap[:1]) + [[BS * 4, G], [4, BS],
                                                  [1, 4]],
                        )[32 * j:32 * (j + 1)]
                        nc.gpsimd.tensor_mul(
                            xb[32 * j:32 * (j + 1), :, :,
                               4 * j:4 * (j + 1)],
                            xvj, mk)
                    U = upool.tile([128, G, D, N], bf16, tag="U")
                    Us.append(U)
                    for g in range(G):
                        ps = ph1ps.tile([128, D, N], f32, tag="ph1")
                        q, gq = g // GQ, g % GQ
                        nc.tensor.matmul(
                            ps, xb[:, g],
                            w2q[q][:, gq * ND:(gq + 1) * ND],
                            start=True, stop=True)
                        if g % 3 != 2:
                            nc.vector.tensor_copy(U[:, g], ps)
                        else:
                            nc.scalar.copy(U[:, g], ps)

            # ---------------- Routing pools ----------------
            tpool = ctx.enter_context(tc.tile_pool(name="tpool", bufs=1))
            tfpool = ctx.enter_context(tc.tile_pool(name="tfpool", bufs=1))
            blpool = ctx.enter_context(tc.tile_pool(name="blpool", bufs=4))
            smpool = ctx.enter_context(tc.tile_pool(name="smpool", bufs=2))
            cpool = ctx.enter_context(tc.tile_pool(name="cpool", bufs=4))
            obcpool = ctx.enter_context(tc.tile_pool(name="obc", bufs=4))
            tiny = ctx.enter_context(tc.tile_pool(name="tiny", bufs=1))
            outps = ctx.enter_context(
                tc.tile_pool(name="outps", bufs=4, space="PSUM"))
            bcps = ctx.enter_context(
                tc.tile_pool(name="bcps", bufs=2, space="PSUM"))

            bls = [blpool.tile([128, G, N], f32, tag="bl", name=f"bl{s}")
                   for s in range(NSUB)]

            def squash_all(ps_os, r):
                """Lockstep squash across sub-chunks. ps_os[s] is a psum
                [BS, MMB, D, N] of MMB parallel partial sums."""
                scale = 1.0 / N if r == 0 else 1.0
                S = range(NSUB)
                v, vsq, nsq, sq, t1, rec, fac, ov, ob = (
                    {}, {}, {}, {}, {}, {}, {}, {}, {})
                for s in S:
                    v[s] = tiny.tile([BS, D, N], f32, tag=f"v{s}",
                                     name=f"v{r}{s}")
                    nc.vector.tensor_scalar_mul(v[s], ps_os[s][:, 0], scale)
                for k in range(1, MMB):
                    for s in S:
                        nc.vector.scalar_tensor_tensor(
                            v[s], ps_os[s][:, k], scale, v[s],
                            op0=OP.mult, op1=OP.add)
                for s in S:
                    vsq[s] = tiny.tile([BS, D, N], f32, tag=f"vsq{s}",
                                       name=f"vsq{r}{s}")
                    nc.vector.tensor_mul(vsq[s], v[s], v[s])
                for s in S:
                    nsq[s] = tiny.tile([BS, N], f32, tag=f"ns{s}",
                                       name=f"ns{r}{s}")
                    nc.vector.tensor_reduce(
                        nsq[s], vsq[s].transpose([0, 2, 1]),
                        axis=AX.X, op=OP.add)
                for s in S:
                    sq[s] = tiny.tile([BS, N], f32, tag=f"sq{s}",
                                      name=f"sq{r}{s}")
                    nc.scalar.sqrt(sq[s], nsq[s])
                for s in S:
                    t1[s] = tiny.tile([BS, N], f32, tag=f"t1{s}",
                                      name=f"t1{r}{s}")
                    nc.vector.tensor_mul(t1[s], nsq[s], sq[s])
                for s in S:
                    nc.vector.tensor_add(t1[s], t1[s], sq[s])
                for s in S:
                    rec[s] = tiny.tile([BS, N], f32, tag=f"rec{s}",
                                       name=f"rec{r}{s}")
                    nc.vector.reciprocal(rec[s], t1[s])
                for s in S:
                    fac[s] = tiny.tile([BS, N], f32, tag=f"fac{s}",
                                       name=f"fac{r}{s}")
                    nc.vector.tensor_mul(fac[s], nsq[s], rec[s])
                for s in S:
                    ov[s] = tiny.tile([BS, D, N], f32, tag=f"ov{s}",
                                      name=f"ov{r}{s}")
                    nc.vector.tensor_mul(ov[s], v[s], _bcast(fac[s], 1, D))
                if r == 2:
                    for s in S:
                        nc.sync.dma_start(y_d[s], ov[s])
                    return None
                # broadcast: [BS, D, N] -> [128, D, N] via PE mask matmul
                obc = {}
                psb = {}
                for s in S:
                    ob[s] = tiny.tile([BS, D, N], bf16, tag=f"ob{s}",
                                      name=f"ob{r}{s}")
                    nc.vector.tensor_copy(ob[s], ov[s])
                for s in S:
                    psb[s] = bcps.tile([128, D, N], f32, tag="bc",
                                       name=f"bc{r}{s}")
                    nc.tensor.matmul(psb[s], bcmask, ob[s],
                                     start=True, stop=True)
                for s in S:
                    obc[s] = obcpool.tile([128, D, N], bf16, tag="obc",
                                          name=f"obc{r}{s}")
                    nc.vector.tensor_copy(obc[s], psb[s])
                return obc

            AGRW = 10  # (blk, s) pairs in flight per agreement window

            def agreement_all(obcs, first):
                """b_l[s] (+)= sum_d U[s] * obc[s], level-major in windows
                so Pool's in-order dispatch never stalls on its own chain."""
                pairs = [(blk, s) for blk in range(NBLK)
                         for s in range(NSUB)]
                for w0 in range(0, len(pairs), AGRW):
                    win = pairs[w0:w0 + AGRW]
                    t2s, t2fs = {}, {}
                    for blk, s in win:
                        g0 = blk * GBLK
                        ri = (blk * NSUB + s) % 10
                        t2 = tpool.tile([128, GBLK, D, N], bf16,
                                        tag=f"t2_{ri}",
                                        name=f"t2_{w0}_{blk}_{s}")
                        nc.vector.tensor_mul(
                            t2, Us[s][:, g0:g0 + GBLK],
                            _bcast(obcs[s], 1, GBLK))
                        t2s[(blk, s)] = t2
                    for blk, s in win:
                        t2 = t2s[(blk, s)]
                        ri = (blk * NSUB + s) % 10
                        t2f = tfpool.tile([128, GBLK, 8, N], bf16,
                                          tag=f"t2f_{ri}",
                                          name=f"t2f_{w0}_{blk}_{s}")
                        nc.vector.tensor_add(
                            t2f, t2[:, :, 0:8], t2[:, :, 8:16])
                        t2fs[(blk, s)] = t2f
                    for blk, s in win:
                        t2f = t2fs[(blk, s)]
                        nc.vector.tensor_add(
                            t2f[:, :, 0:4], t2f[:, :, 0:4], t2f[:, :, 4:8])
                    for blk, s in win:
                        t2f = t2fs[(blk, s)]
                        nc.gpsimd.tensor_add(
                            t2f[:, :, 0:2], t2f[:, :, 0:2], t2f[:, :, 2:4])
                    for blk, s in win:
                        t2f = t2fs[(blk, s)]
                        g0 = blk * GBLK
                        if first:
                            nc.gpsimd.tensor_add(
                                bls[s][:, g0:g0 + GBLK],
                                t2f[:, :, 0], t2f[:, :, 1])
                        else:
                            nc.gpsimd.tensor_add(
                                t2f[:, :, 0], t2f[:, :, 0], t2f[:, :, 1])
                            nc.gpsimd.tensor_add(
                                bls[s][:, g0:g0 + GBLK],
                                bls[s][:, g0:g0 + GBLK], t2f[:, :, 0])

            # ---- r=0: c uniform -> outputs = mean_i u_hat (PE only) ----
            ps_os = {}
            for s in range(NSUB):
                ps_o = outps.tile([BS, MMB, D, N], f32, tag="po",
                                  name=f"po0_{s}")
                for j in range(G // MMB):
                    nc.tensor.matmul(
                        ps_o, bmask, Us[s][:, j * MMB:(j + 1) * MMB],
                        start=(j == 0), stop=(j == G // MMB - 1))
                ps_os[s] = ps_o
            obcs = squash_all(ps_os, 0)
            agreement_all(obcs, first=True)

            # ---- r = 1, 2 ----
            for r in (1, 2):
                es, zs, rzs, cs = {}, {}, {}, {}
                for s in range(NSUB):
                    es[s] = smpool.tile([128, G, N], f32, tag="e",
                                        name=f"e{r}{s}")
                    nc.scalar.activation(es[s], bls[s], AF.Exp)
                for s in range(NSUB):
                    zs[s] = smpool.tile([128, G], f32, tag="z",
                                        name=f"z{r}{s}")
                    nc.vector.tensor_reduce(
                        zs[s], es[s], axis=AX.X, op=OP.add)
                for s in range(NSUB):
                    rzs[s] = smpool.tile([128, G], f32, tag="rz",
                                         name=f"rz{r}{s}")
                    nc.vector.reciprocal(rzs[s], zs[s])
                for s in range(NSUB):
                    cs[s] = cpool.tile([128, G, N], bf16, tag="c",
                                       name=f"c{r}{s}")
                    nc.vector.tensor_mul(cs[s], es[s], _bcast(rzs[s], 2, N))

                ps_os = {}
                for s in range(NSUB):
                    ps_os[s] = outps.tile([BS, MMB, D, N], f32, tag="po",
                                          name=f"po{r}_{s}")
                for blk in range(NBLK):
                    for s in range(NSUB):
                        g0 = blk * GBLK
                        tm = tpool.tile([128, GBLK, D, N], bf16,
                                        tag=f"t2_{(blk * NSUB + s) % 10}",
                                        name=f"tm{r}_{s}_{blk}")
                        nc.vector.tensor_mul(
                            tm, Us[s][:, g0:g0 + GBLK],
                            _bcast(cs[s][:, g0:g0 + GBLK], 2, D))
                        for q in range(GBLK // MMB):
                            j = blk * (GBLK // MMB) + q
                            nc.tensor.matmul(
                                ps_os[s], bmask,
                                tm[:, q * MMB:(q + 1) * MMB],
                                start=(j == 0),
                                stop=(j == G // MMB - 1),
                                skip_group_check=True)
                obcs = squash_all(ps_os, r)
                if r == 1:
                    agreement_all(obcs, first=False)
    _legalize_waits(nc)
    return nc


def _prep_inputs(inputs, W):
    """Host-side layout prep. Returns per-core input maps."""
    W = np.asarray(W, dtype=np.float32)
    inputs = np.asarray(inputs, dtype=np.float32)
    # W2[(ig,k), (g,d,n)] = W[n, 16g+ig, d, k]
    Wr = W.reshape(N, G, IG, D, DK)
    w2 = np.ascontiguousarray(
        Wr.transpose(2, 4, 1, 3, 0)).reshape(128, G * ND).astype(BF16)
    bmask = np.repeat(np.eye(BS, dtype=np.float32), IG, axis=0).astype(BF16)
    bcmask = np.repeat(np.eye(BS, dtype=np.float32), IG, axis=1).astype(BF16)

    # delta-mask over the 4-wide near-diagonal: dmask[(ig,k), t] = (ig%4==t)
    ig_idx = np.arange(128) // DK          # ig of partition (ig,k)
    dmask = (ig_idx[:, None] % 4 == np.arange(4)[None, :]).astype(np.float32)

    in_maps = []
    for cc in range(NCORES):
        xcore = inputs[cc * BC:(cc + 1) * BC]       # [32, 1152, 8]
        xr = xcore.reshape(NSUB, BS, G, IG, DK)     # [s, b, g, ig, k]
        # xq[(ig,k), s, g, b, t] = x[s*8+b, 16g + 4*(ig//4) + t, k]
        # (unmasked; the on-chip dmask multiply zeroes ig%4 != t)
        xq = np.empty((IG, DK, NSUB, G, BS, 4), np.float32)
        for t in range(4):
            v = xr[:, :, :, t::4, :]                # [s, b, g, ig4=4, k]
            xq[:, :, :, :, :, t] = np.repeat(
                v.transpose(3, 4, 0, 2, 1), 4, axis=0)  # [ig, k, s, g, b]
        xq = xq.reshape(128, NSUB, G * BS * 4)
        # append the mask to each sub-chunk slice (keeps the slice's DMA
        # self-contained so the on-chip consumer waits on one queue only)
        xq = np.concatenate(
            [xq, np.broadcast_to(dmask[:, None, :], (128, NSUB, 4))],
            axis=2).astype(BF16)
        in_maps.append(
            {"xc": xq, "w2": w2, "bmask": bmask, "bcmask": bcmask})
    return in_maps


def _run(inputs, W, trace=False):
    from concourse.bass_utils import run_bass_kernel_spmd

    if "nc" not in _cache:
        _cache["nc"] = _build_nc()
    nc = _cache["nc"]
    in_maps = _prep_inputs(inputs, W)
    res = run_bass_kernel_spmd(
        nc, in_maps, core_ids=list(range(NCORES)), trace=trace)
    # y[s, b, (d, n)] per core -> out[b_global, n, d]
    out = np.empty((B, N, D), np.float32)
    for cc in range(NCORES):
        yc = res.results[cc]["y"].reshape(NSUB, BS, D, N)
        out[cc * BC:(cc + 1) * BC] = yc.transpose(0, 1, 3, 2).reshape(
            BC, N, D)
    return out, res


def kernel(inputs, W):
    out, _ = _run(inputs, W, trace=False)
    return out



# revision 30
# speedup vs baseline: 1.3875x; 1.3875x over previous
"""CapsuleLayer (dynamic routing) Trainium2 Bass kernel, v2.

Sharding: pure data-parallel over batch B=256 -> 8 cores x 32 batches.
Per core 32 batches run as 4 sub-chunks of 8; SBUF partition dim for
u_hat packs p = b*16 + ig where capsule i = 16*g + ig, g in [0,72).

Phase A (u_hat): K=8 contraction packed to K=128 by block-diagonalizing
16 capsules into the PE stationary. The block-diagonal stationary
(zeros included) is pre-built on the host and DMAd straight into SBUF,
so no compute engine touches the expansion:
    xq[(ig,k), s, (g, b, ig')] = x[32c+8s+b, 16g+ig', k] * (ig'==ig)
    psum[(b,ig), (d,n)] = u_hat[8s+b, n, 16g+ig, d]
U stays on-chip in bf16 as U[s][128, G, D, N] (n minor so broadcast
multiplies keep a unit-stride innermost axis for the DVE 2x mode).

Round 0 outputs bypass U entirely (x-space): sum_i u_hat[b,n,i,d]
= sum_{(i,k)} x[b,i,k] W[n,i,d,k], a dense 9216-deep contraction done
as 72 PE matmuls with a dense x stationary covering all 32 batches.
This removes the phaseA->routing barrier: squash(r0) completes while
phase A is still copying U.

Routing: reduce-matmuls use 32-wide block masks (bm[p, 8s+p//16]) so
all 4 sub-chunks accumulate into ONE [32, MMB, D, N] psum -> a single
squash per round on 32 partitions. Agreement = DVE multiply (obc bcast
over g) + in-place fold tree over d (L1-L3 on DVE at 2x, L4+bl on
Pool). Softmax: ACT exp (bf16), DVE reduce/recip/scale.

Engine budget (cost model): DVE ~140us (4 full-U multiplies + folds),
ACT ~50us (U psum->sbuf copies, exp), Pool ~60us (copy share, fold
tail), PE ~82us, DMA ~37us. Emission is stage-major (lockstep) across
sub-chunks so each engine queue stays dense.
"""

import numpy as np
import ml_dtypes

B, N, I, D, DK = 256, 10, 1152, 16, 8
NCORES = 8
BC = B // NCORES      # 32 batches per core
BS = 8                # batches per sub-chunk
NSUB = BC // BS       # 4
IG = 16               # capsules per PE group
G = I // IG           # 72
ND = D * N            # 160 (d-major, n-minor)
GQ = 18               # g per w2 quarter
GB = 24               # g per routing block
NBLK = G // GB        # 3
MMB = 3               # g per reduce-matmul (480 moving cols)
BF16 = ml_dtypes.bfloat16

_cache = {}


def _bcast(ap, axis, count):
    """Insert a stride-0 dim of size `count` at `axis`."""
    ap = ap.unsqueeze(axis)
    shape = list(ap.shape)
    shape[axis] = count
    return ap.broadcast_to(shape)


def _legalize_waits(nc):
    """This walrus build takes at most 1 embedded sync wait per TPB
    instruction (2 on EventSemaphore, 0 on Drain). Tile emits multi-wait
    sync_info; hoist the extras onto preceding EventSemaphore instructions
    on the same engine queue."""
    from concourse import mybir

    n = 0
    for fn in nc.m.functions:
        for blk in fn.blocks:
            out = []
            for inst in blk.instructions:
                si = inst.sync_info
                if si is not None and si.on_wait:
                    keep = 1
                    if inst.opcode == "Drain":
                        keep = 0
                    elif inst.opcode == "EventSemaphore":
                        keep = 2
                    w = list(si.on_wait)
                    if len(w) > keep:
                        extra = w[:len(w) - keep] if keep else w
                        kept = w[len(w) - keep:] if keep else []
                        for i0 in range(0, len(extra), 2):
                            n += 1
                            out.append(mybir.InstEventSemaphore(
                                name=f"{inst.name}-hw{n}",
                                engine=inst.engine, ins=[], outs=[],
                                sync_info=mybir.SyncInfo(
                                    on_wait=extra[i0:i0 + 2],
                                    on_update=[]),
                            ))
                        si.on_wait = kept
                out.append(inst)
            blk.instructions = out
    return n


def _build_nc():
    import concourse.bass as bass
    import concourse.tile as tile
    from concourse import mybir
    from contextlib import ExitStack

    f32 = mybir.dt.float32
    bf16 = mybir.dt.bfloat16
    AX = mybir.AxisListType
    OP = mybir.AluOpType
    AF = mybir.ActivationFunctionType

    nc = bass.Bass()
    xq_d = nc.dram_tensor("xq", [128, NSUB, G * BS * IG], bf16,
                          kind="ExternalInput")
    w2_d = nc.dram_tensor("w2", [128, G * ND], bf16, kind="ExternalInput")
    xg_d = nc.dram_tensor("xg", [128, G * BC], bf16, kind="ExternalInput")
    bm_d = nc.dram_tensor("bm", [128, NSUB * BC], bf16,
                          kind="ExternalInput")
    bc_d = nc.dram_tensor("bc", [BC, NSUB * 128], bf16,
                          kind="ExternalInput")
    y_d = nc.dram_tensor("y", [BC, ND], f32, kind="ExternalOutput")

    with tile.TileContext(nc) as tc:
        with ExitStack() as ctx:
            singles = ctx.enter_context(tc.tile_pool(name="singles", bufs=1))

            # One ACT hwdge queue for the input DMAs: the modeled DMA
            # device serializes transfers, so emission order IS the
            # arrival order. xq0 goes right after w2q0 so phase A's
            # first g-quarter can start while the w2 tail streams.
            xg = singles.tile([128, G, BC], bf16, tag="xg")
            nc.scalar.dma_start(xg, xg_d[:])
            w2q = [singles.tile([128, GQ * ND], bf16, tag=f"w2_{q}",
                                name=f"w2_{q}") for q in range(4)]
            nc.scalar.dma_start(w2q[0], w2_d[:, 0:GQ * ND])

            upool = ctx.enter_context(tc.tile_pool(name="upool", bufs=1))
            Us = [upool.tile([128, G, D, N], bf16, tag=f"U{s}",
                             name=f"U{s}")
                  for s in range(NSUB)]
            blpool = ctx.enter_context(tc.tile_pool(name="blpool", bufs=1))
            bls = [blpool.tile([128, G, N], f32, tag=f"bl{s}",
                               name=f"bl{s}")
                   for s in range(NSUB)]
            r0ps_pool = ctx.enter_context(
                tc.tile_pool(name="r0ps", bufs=1, space="PSUM"))
            r0ps = r0ps_pool.tile([BC, D, N], f32, tag="r0", name="r0ps")
            obcps = ctx.enter_context(
                tc.tile_pool(name="obcps", bufs=2, space="PSUM"))
            obcpool = ctx.enter_context(tc.tile_pool(name="obc", bufs=2))
            tiny = ctx.enter_context(tc.tile_pool(name="tiny", bufs=2))

            t2pool = ctx.enter_context(
                tc.tile_pool(name="t2pool", bufs=3))

            # ---------------- Phase A + r0 outputs ----------------
            with ExitStack() as actx:
                xbpool = actx.enter_context(
                    tc.tile_pool(name="xbpool", bufs=2))
                ph1ps = actx.enter_context(
                    tc.tile_pool(name="ph1ps", bufs=5, space="PSUM"))

                # DMA arrival order (all on ACT's queue except xq2/xq3,
                # whose SP dispatch is held back naturally by the xb
                # buffer rotation): xg, w2q0, xq0, w2q1-3, bm, bc, xq1
                xbs = []
                xb0 = xbpool.tile([128, G, BS * IG], bf16, tag="xb",
                                  name="xb0")
                nc.scalar.dma_start(xb0, xq_d[:, 0])
                xbs.append(xb0)
                for q in range(1, 4):
                    nc.scalar.dma_start(
                        w2q[q], w2_d[:, q * GQ * ND:(q + 1) * GQ * ND])
                bm = singles.tile([128, NSUB, BC], bf16, tag="bm")
                nc.scalar.dma_start(bm, bm_d[:])
                bcm = singles.tile([BC, NSUB, 128], bf16, tag="bc")
                nc.scalar.dma_start(bcm, bc_d[:])
                xb1 = xbpool.tile([128, G, BS * IG], bf16, tag="xb",
                                  name="xb1")
                nc.scalar.dma_start(xb1, xq_d[:, 1])
                xbs.append(xb1)
                for s in (2, 3):
                    xb = xbpool.tile([128, G, BS * IG], bf16, tag="xb",
                                     name=f"xb{s}")
                    nc.sync.dma_start(xb, xq_d[:, s])
                    xbs.append(xb)

                # copies: ACT + a DVE share; Pool stays clear for the
                # agreement fold tail (in-order queues: a Pool copy
                # backlog would block bl updates behind it)
                cpeng = [
                    lambda o, i: nc.scalar.copy(o, i),
                    lambda o, i: nc.scalar.copy(o, i),
                    lambda o, i: nc.scalar.copy(o, i),
                    lambda o, i: nc.vector.tensor_copy(o, i),
                ]

                def phA(s, j):
                    ps = ph1ps.tile([128, MMB, D, N], f32,
                                    tag="ph1", name=f"ph{s}_{j}")
                    for m in range(MMB):
                        g = j * MMB + m
                        nc.tensor.matmul(
                            ps[:, m], xbs[s][:, g],
                            w2q[g // GQ][:, (g % GQ) * ND:
                                         (g % GQ + 1) * ND],
                            start=True, stop=True,
                            skip_group_check=True)
                    cpeng[j % 4](Us[s][:, j * MMB:(j + 1) * MMB], ps)

                # s0 interleaves with the r0 x-space matmuls per w2
                # quarter so each starts as soon as its DMAs land
                for q in range(4):
                    for gq in range(GQ):
                        g = q * GQ + gq
                        nc.tensor.matmul(
                            r0ps, xg[:, g],
                            w2q[q][:, gq * ND:(gq + 1) * ND],
                            start=(g == 0), stop=(g == G - 1),
                            skip_group_check=True)
                    for j in range(q * 6, (q + 1) * 6):
                        phA(0, j)

                # squash r0 now (c uniform -> mean via the x-space psum):
                # its ACT/DVE ops must queue BEFORE s1-s3's copies or the
                # whole routing chain waits on the copy backlog
                obc0 = _squash32(nc, tc, tiny, obcpool, obcps, bcm,
                                 r0ps, None, 0, y_d, AX, OP, AF, f32, bf16)

                # interleave the remaining phase-A chunks with the r0
                # agreement windows so each engine's in-order queue
                # matches data arrival (agr0[s] right after U[s+1]'s
                # copies are queued; U[s] is ready by then)
                for s in range(1, NSUB):
                    for j in range(24):
                        phA(s, j)
                    _agr_win(nc, t2pool, Us, bls, obc0, s - 1, True)
                _agr_win(nc, t2pool, Us, bls, obc0, NSUB - 1, True)

            # pools for routing (reuse SBUF freed by xbpool)
            tmpool = ctx.enter_context(tc.tile_pool(name="tmpool", bufs=3))
            espool = ctx.enter_context(tc.tile_pool(name="espool", bufs=2))
            rps = ctx.enter_context(
                tc.tile_pool(name="rps", bufs=2, space="PSUM"))

            # ---- rounds 1, 2 ----
            for r in (1, 2):
                es = []
                for s in range(NSUB):
                    e = espool.tile([128, G, N], bf16, tag=f"es{s}",
                                    name=f"es{r}{s}")
                    nc.scalar.activation(e, bls[s], AF.Exp)
                    es.append(e)
                zs, rzs = [], []
                for s in range(NSUB):
                    z = tiny.tile([128, G], f32, tag=f"z{s}",
                                  name=f"z{r}{s}")
                    nc.vector.tensor_reduce(z, es[s], axis=AX.X,
                                            op=OP.add)
                    zs.append(z)
                for s in range(NSUB):
                    rz = tiny.tile([128, G], f32, tag=f"rz{s}",
                                   name=f"rz{r}{s}")
                    nc.vector.reciprocal(rz, zs[s])
                    rzs.append(rz)
                for s in range(NSUB):
                    nc.vector.tensor_mul(es[s], es[s],
                                         _bcast(rzs[s], 2, N))

                psr = rps.tile([BC, MMB, D, N], f32, tag="rr",
                               name=f"rr{r}")
                for blk in range(NBLK):
                    for s in range(NSUB):
                        g0 = blk * GB
                        tm = tmpool.tile([128, GB, D, N], bf16, tag="tm",
                                         name=f"tm{r}_{blk}_{s}")
                        nc.vector.tensor_mul(
                            tm, Us[s][:, g0:g0 + GB],
                            _bcast(es[s][:, g0:g0 + GB], 2, D))
                        for q in range(GB // MMB):
                            nc.tensor.matmul(
                                psr, bm[:, s],
                                tm[:, q * MMB:(q + 1) * MMB],
                                start=(blk == 0 and s == 0 and q == 0),
                                stop=(blk == NBLK - 1 and s == NSUB - 1
                                      and q == GB // MMB - 1),
                                skip_group_check=True)
                obc = _squash32(nc, tc, tiny, obcpool, obcps, bcm,
                                psr, MMB, r, y_d, AX, OP, AF, f32, bf16)
                if r == 1:
                    for s in range(NSUB):
                        _agr_win(nc, t2pool, Us, bls, obc, s, False)
    _legalize_waits(nc)
    return nc


def _squash32(nc, tc, tiny, obcpool, obcps, bcm, ps, mmb, r, y_d,
              AX, OP, AF, f32, bf16):
    """Squash on the 32-wide psum; returns per-sub-chunk obc (bf16,
    broadcast to 128 partitions) or None for the last round."""
    OP = __import__("concourse.mybir", fromlist=["x"]).AluOpType
    v = tiny.tile([BC, D, N], f32, tag="v", name=f"v{r}")
    if mmb is None:
        nc.vector.tensor_scalar_mul(v, ps, 1.0 / N)
    else:
        # one PSUM operand per instruction (walrus rejects 2-PSUM TT)
        nc.vector.tensor_scalar_mul(v, ps[:, 0], 1.0)
        for m in range(1, mmb):
            nc.vector.scalar_tensor_tensor(v, ps[:, m], 1.0, v,
                                           op0=OP.mult, op1=OP.add)
    vsq = tiny.tile([BC, D, N], f32, tag="vsq", name=f"vsq{r}")
    nc.scalar.activation(vsq, v, AF.Square)
    nsq = tiny.tile([BC, N], f32, tag="nsq", name=f"nsq{r}")
    nc.vector.tensor_reduce(nsq, vsq.transpose([0, 2, 1]),
                            axis=AX.X, op=OP.add)
    sq = tiny.tile([BC, N], f32, tag="sq", name=f"sq{r}")
    nc.scalar.sqrt(sq, nsq)
    t1 = tiny.tile([BC, N], f32, tag="t1", name=f"t1{r}")
    nc.vector.tensor_mul(t1, nsq, sq)
    nc.vector.tensor_add(t1, t1, sq)
    rec = tiny.tile([BC, N], f32, tag="rec", name=f"rec{r}")
    nc.vector.reciprocal(rec, t1)
    fac = tiny.tile([BC, N], f32, tag="fac", name=f"fac{r}")
    nc.vector.tensor_mul(fac, nsq, rec)
    if r == 2:
        ov = tiny.tile([BC, D, N], f32, tag="ov", name=f"ov{r}")
        nc.vector.tensor_mul(ov, v, _bcast(fac, 1, D))
        nc.sync.dma_start(y_d[:], ov)
        return None
    ovb = tiny.tile([BC, D, N], bf16, tag="ovb", name=f"ovb{r}")
    nc.vector.tensor_mul(ovb, v, _bcast(fac, 1, D))
    obcs = []
    for s in range(NSUB):
        psb = obcps.tile([128, D, N], f32, tag="bc", name=f"bc{r}{s}")
        nc.tensor.matmul(psb, bcm[:, s], ovb, start=True, stop=True,
                         skip_group_check=True)
        ob = obcpool.tile([128, D, N], bf16, tag=f"obc{s}",
                          name=f"obc{r}{s}")
        nc.scalar.copy(ob, psb)
        obcs.append(ob)
    return obcs


def _agr_win(nc, t2pool, Us, bls, obcs, s, first):
    """One sub-chunk of agreement: bl[s] (+)= sum_d U[s] * obc[s].
    Stage-major over the NBLK g-blocks; in-place fold tree over d
    (L1-L3 on DVE at 2x, L4 + bl accumulate on Pool)."""
    from concourse import mybir
    bf16 = mybir.dt.bfloat16
    rtag = "a0" if first else "a1"
    obc = obcs[s]
    t2s = []
    for blk in range(NBLK):
        g0 = blk * GB
        t2 = t2pool.tile([128, GB, D, N], bf16, tag="t2",
                         name=f"t2{rtag}_{blk}_{s}")
        nc.vector.tensor_mul(t2, Us[s][:, g0:g0 + GB],
                             _bcast(obc, 1, GB))
        t2s.append(t2)
    for t2 in t2s:  # L1: 16 -> 8
        nc.vector.tensor_add(t2[:, :, 0:8], t2[:, :, 0:8], t2[:, :, 8:16])
    for t2 in t2s:  # L2: 8 -> 4
        nc.vector.tensor_add(t2[:, :, 0:4], t2[:, :, 0:4], t2[:, :, 4:8])
    for t2 in t2s:  # L3: 4 -> 2
        nc.vector.tensor_add(t2[:, :, 0:2], t2[:, :, 0:2], t2[:, :, 2:4])
    for blk, t2 in enumerate(t2s):  # L4 + bl accumulate on Pool
        g0 = blk * GB
        if first:
            nc.gpsimd.tensor_add(bls[s][:, g0:g0 + GB],
                                 t2[:, :, 0], t2[:, :, 1])
        else:
            nc.gpsimd.tensor_add(t2[:, :, 0], t2[:, :, 0], t2[:, :, 1])
            nc.gpsimd.tensor_add(bls[s][:, g0:g0 + GB],
                                 bls[s][:, g0:g0 + GB], t2[:, :, 0])


def _prep_inputs(inputs, W):
    """Host-side layout prep. Returns per-core input maps."""
    W = np.asarray(W, dtype=np.float32)
    inputs = np.asarray(inputs, dtype=np.float32)
    # w2[(ig,k), (g,d,n)] = W[n, 16g+ig, d, k]
    Wr = W.reshape(N, G, IG, D, DK)
    w2 = np.ascontiguousarray(
        Wr.transpose(2, 4, 1, 3, 0)).reshape(128, G * ND).astype(BF16)
    # bm[(b,ig), s, b32] = (b32 == 8s + b);  p' = b*16 + ig
    pb = np.arange(128) // IG
    bm = (np.arange(BC)[None, None, :]
          == (8 * np.arange(NSUB)[None, :, None] + pb[:, None, None])
          ).astype(np.float32).astype(BF16)
    # bc[b32, s, p'] = (b32 == 8s + p'//16)
    bc = (np.arange(BC)[:, None, None]
          == (8 * np.arange(NSUB)[None, :, None] + pb[None, None, :])
          ).astype(np.float32).astype(BF16)

    ii = np.arange(IG)
    in_maps = []
    for cc in range(NCORES):
        xcore = inputs[cc * BC:(cc + 1) * BC]          # [32, 1152, 8]
        xr = xcore.reshape(NSUB, BS, G, IG, DK)        # [s, b, g, ig', k]
        # xq[(ig,k), s, (g, b, ig')] block-diagonal with explicit zeros
        xq = np.zeros((IG, DK, NSUB, G, BS, IG), np.float32)
        xq[ii, :, :, :, :, ii] = xr.transpose(3, 4, 0, 2, 1)
        xq = xq.reshape(128, NSUB, G * BS * IG).astype(BF16)
        # xg[(ig,k), g, b32] dense (for the x-space r0 contraction)
        xg = np.ascontiguousarray(
            xcore.reshape(BC, G, IG, DK).transpose(2, 3, 1, 0)
        ).reshape(128, G * BC).astype(BF16)
        in_maps.append({"xq": xq, "w2": w2, "xg": xg,
                        "bm": bm.reshape(128, NSUB * BC),
                        "bc": bc.reshape(BC, NSUB * 128)})
    return in_maps


def _run(inputs, W, trace=False):
    from concourse.bass_utils import run_bass_kernel_spmd

    if "nc" not in _cache:
        _cache["nc"] = _build_nc()
    nc = _cache["nc"]
    in_maps = _prep_inputs(inputs, W)
    res = run_bass_kernel_spmd(
        nc, in_maps, core_ids=list(range(NCORES)), trace=trace)
    # y[b32, (d, n)] per core -> out[b_global, n, d]
    out = np.empty((B, N, D), np.float32)
    for cc in range(NCORES):
        yc = res.results[cc]["y"].reshape(BC, D, N)
        out[cc * BC:(cc + 1) * BC] = yc.transpose(0, 2, 1)
    return out, res


def kernel(inputs, W):
    out, _ = _run(inputs, W, trace=False)
    return out
